# revision 2
# baseline (speedup 1.0000x reference)
"""MlssaSelector Trainium2 kernel (8-core SPMD, data-parallel over bags).

Pipeline per core (16384 sentences = 1024 bags, bag boundaries shard-aligned):
  sT  = W1.T @ xT            (PE, bf16, transposed domain: partitions = d/a)
  th  = tanh(sT)             (ACT, PSUM->SBUF, bf16 out)
  zT  = W2.T @ th            (PE)  -> [4 heads, n]
  e   = exp(zT)              (ACT; segment softmax needs no max-subtraction:
                              |z| <~ 3 so exp is safe in f32)
  den = segsum_16(e), rden = 1/den               (DVE, free-dim grouped reduce)
  wraw[n,b] = sum_h e[h,n] * rden[h,b]           (PE, K=4 matmul -> crosses w
                                                  from free-dim to partitions)
  wsel = 0.25 * bagmask * wraw                   (DVE; 0.25 folds head-mean)
  bagT[d,b] = x_nat.T-chunks @ wsel              (PE, x as stationary operand:
                                                  output is transposed directly)
  logits = bagT.T @ Wc + bc                      (PE + DVE bias add)
x is shipped in BOTH layouts as bf16 (16.75MB each) so total HBM traffic per
core stays at the 33.5MB f32-once roofline (~94us @ 358GB/s).
"""

import numpy as np
import ml_dtypes

import concourse.bacc as bacc
import concourse.mybir as mybir
import concourse.tile as tile
from concourse import bass_utils

BF16 = ml_dtypes.bfloat16

N = 131072
D = 512
A = 256          # D_ATT
H = 4            # heads
C = 53           # classes
BAG = 16
NCORES = 8
NSH = N // NCORES          # 16384 sentences per core
BSH = NSH // BAG           # 1024 bags per core
NT = 512                   # sentences per supertile
NSUP = NSH // NT           # 32 supertiles
SUBS = NT // 128           # 4 x 128-sentence subtiles per supertile
BAGS_PER_SUP = NT // BAG   # 32

_CACHE = {}


def _build_nc():
    nc = bacc.Bacc("TRN2", target_bir_lowering=False, debug=False)
    dt = mybir.dt

    xt_d = nc.dram_tensor("xt", [D, NSH], dt.bfloat16, kind="ExternalInput")
    xn_d = nc.dram_tensor("xn", [NSH, D], dt.bfloat16, kind="ExternalInput")
    w1_d = nc.dram_tensor("w1", [D, A], dt.bfloat16, kind="ExternalInput")
    w2_d = nc.dram_tensor("w2", [A, H], dt.bfloat16, kind="ExternalInput")
    wc_d = nc.dram_tensor("wc", [D, C], dt.bfloat16, kind="ExternalInput")
    m0_d = nc.dram_tensor("m0", [128, SUBS * 8], dt.float32, kind="ExternalInput")
    bc_d = nc.dram_tensor("bc", [128, C], dt.float32, kind="ExternalInput")
    out_d = nc.dram_tensor("logits", [BSH, C], dt.float32, kind="ExternalOutput")

    xt_r = xt_d[:].rearrange("(dc p) n -> p dc n", p=128)          # [128,4,NSH]
    xn_r = xn_d[:].rearrange("(j s p) d -> j p s d", p=128, s=SUBS)  # [32,128,4,512]
    out_r = out_d[:].rearrange("(t p) c -> p t c", p=128)          # [128,8,53]

    with tile.TileContext(nc) as tc:
        with (
            tc.tile_pool(name="consts", bufs=1) as consts,
            tc.tile_pool(name="xin", bufs=3) as xin,
            tc.tile_pool(name="acts", bufs=4) as acts,
            tc.tile_pool(name="small", bufs=3) as small,
            tc.tile_pool(name="persist", bufs=1) as persist,
            tc.tile_pool(name="ps_s", bufs=2, space="PSUM") as ps_s_pool,
            tc.tile_pool(name="ps_z", bufs=2, space="PSUM") as ps_z_pool,
            tc.tile_pool(name="ps_w", bufs=2, space="PSUM") as ps_w_pool,
            tc.tile_pool(name="ps_bag", bufs=2, space="PSUM") as ps_bag_pool,
        ):
            # ---- constants / weights ----
            w1_sb = consts.tile([128, 4, A], dt.bfloat16)
            nc.sync.dma_start(out=w1_sb, in_=w1_d[:].rearrange("(dc p) a -> p dc a", p=128))
            w2_sb = consts.tile([128, 2, H], dt.bfloat16)
            nc.sync.dma_start(out=w2_sb, in_=w2_d[:].rearrange("(ah p) h -> p ah h", p=128))
            wc_sb = consts.tile([128, 4, C], dt.bfloat16)
            nc.sync.dma_start(out=wc_sb, in_=wc_d[:].rearrange("(dc p) c -> p dc c", p=128))
            m0_sb = consts.tile([128, SUBS * 8], dt.float32)
            nc.sync.dma_start(out=m0_sb, in_=m0_d[:])
            bc_sb = consts.tile([128, C], dt.float32)
            nc.sync.dma_start(out=bc_sb, in_=bc_d[:])

            # transposed bag_repr accumulated over the whole shard: [d-part, dc, b]
            bagT = persist.tile([128, 4, BSH], dt.bfloat16)

            for j in range(NSUP):
                xt_t = xin.tile([128, 4, NT], dt.bfloat16, tag="xt")
                nc.sync.dma_start(out=xt_t, in_=xt_r[:, :, j * NT:(j + 1) * NT])
                xn_t = xin.tile([128, SUBS, D], dt.bfloat16, tag="xn")
                nc.sync.dma_start(out=xn_t, in_=xn_r[j])

                # stage 1: sT = W1.T @ xT, tanh
                th_t = acts.tile([128, 2, NT], dt.bfloat16, tag="th")
                for ah in range(2):
                    ps_s = ps_s_pool.tile([128, NT], dt.float32, tag="s")
                    for dc in range(4):
                        nc.tensor.matmul(
                            ps_s,
                            w1_sb[:, dc, ah * 128:(ah + 1) * 128],
                            xt_t[:, dc, :],
                            start=dc == 0,
                            stop=dc == 3,
                        )
                    nc.scalar.activation(
                        th_t[:, ah, :], ps_s, mybir.ActivationFunctionType.Tanh
                    )

                # stage 2: zT = W2.T @ th  -> [4, NT]
                ps_z = ps_z_pool.tile([H, NT], dt.float32, tag="zz")
                for ah in range(2):
                    nc.tensor.matmul(
                        ps_z, w2_sb[:, ah, :], th_t[:, ah, :],
                        start=ah == 0, stop=ah == 1,
                    )

                # segment softmax (no max): e, den, rden
                e_t = acts.tile([H, NT], dt.bfloat16, tag="e")
                nc.scalar.activation(e_t, ps_z, mybir.ActivationFunctionType.Exp)
                den = small.tile([H, BAGS_PER_SUP], dt.float32, tag="den")
                nc.vector.tensor_reduce(
                    den, e_t.rearrange("h (b i) -> h b i", i=BAG),
                    axis=mybir.AxisListType.X, op=mybir.AluOpType.add,
                )
                rden = small.tile([H, BAGS_PER_SUP], dt.float32, tag="rden")
                nc.vector.reciprocal(rden, den)
                rdbf = small.tile([H, BAGS_PER_SUP], dt.bfloat16, tag="rdbf")
                nc.vector.tensor_copy(rdbf, rden)

                # wraw[n, b] = sum_h e[h, n] * rden[h, b]   (per 128-subtile)
                ps_w = ps_w_pool.tile([128, SUBS, 8], dt.float32, tag="w")
                for s in range(SUBS):
                    nc.tensor.matmul(
                        ps_w[:, s, :],
                        e_t[:, s * 128:(s + 1) * 128],
                        rdbf[:, s * 8:(s + 1) * 8],
                        start=True, stop=True,
                    )
                wsel = acts.tile([128, SUBS * 8], dt.bfloat16, tag="wsel")
                nc.vector.tensor_tensor(
                    out=wsel, in0=m0_sb,
                    in1=ps_w.rearrange("p s b -> p (s b)"),
                    op=mybir.AluOpType.mult,
                )

                # bagT[d, b] = sum_n x[n, d] * wsel[n, b]  (x stationary)
                ps_b = ps_bag_pool.tile([128, 4, BAGS_PER_SUP], dt.float32, tag="bag")
                for s in range(SUBS):
                    for dc in range(4):
                        nc.tensor.matmul(
                            ps_b[:, dc, 8 * s:8 * s + 8],
                            xn_t[:, s, dc * 128:(dc + 1) * 128],
                            wsel[:, 8 * s:8 * s + 8],
                            start=True, stop=True,
                        )
                nc.any.tensor_copy(bagT[:, :, j * BAGS_PER_SUP:(j + 1) * BAGS_PER_SUP], ps_b)

            # ---- tail: logits = bagT.T @ Wc + bc ----
            log_sb = persist.tile([128, BSH // 128, C], dt.float32)
            for g in range(BSH // 128):
                ps_l = ps_z_pool.tile([128, C], dt.float32, tag="zz")
                for dc in range(4):
                    nc.tensor.matmul(
                        ps_l, bagT[:, dc, g * 128:(g + 1) * 128], wc_sb[:, dc, :],
                        start=dc == 0, stop=dc == 3,
                    )
                nc.vector.tensor_tensor(
                    out=log_sb[:, g, :], in0=ps_l, in1=bc_sb,
                    op=mybir.AluOpType.add,
                )
            nc.sync.dma_start(out=out_r, in_=log_sb)

    nc.compile()
    return nc


def _get_nc():
    if "nc" not in _CACHE:
        _CACHE["nc"] = _build_nc()
    return _CACHE["nc"]


def kernel(x, W1, W2, Wc, bc, seg_ids, n_bags, _trace=False):
    x = np.asarray(x, dtype=np.float32)
    W1 = np.asarray(W1, dtype=np.float32)
    W2 = np.asarray(W2, dtype=np.float32)
    Wc = np.asarray(Wc, dtype=np.float32)
    bc = np.asarray(bc, dtype=np.float32)

    w1_b = W1.astype(BF16)
    w2_b = W2.astype(BF16)
    wc_b = Wc.astype(BF16)
    bc_b = np.broadcast_to(bc, (128, C)).copy()
    # bag-membership mask scaled by 1/H (folds the head-mean)
    p = np.arange(128)
    m0 = np.zeros((128, SUBS * 8), np.float32)
    for s in range(SUBS):
        m0[np.arange(128), s * 8 + (p // BAG) % 8] = 0.25
    ident = np.eye(128, dtype=np.float32)

    nc = _get_nc()
    in_maps = []
    for c in range(NCORES):
        xs = x[c * NSH:(c + 1) * NSH]
        in_maps.append({
            "xt": np.ascontiguousarray(xs.T).astype(BF16),
            "xn": xs.astype(BF16),
            "w1": w1_b, "w2": w2_b, "wc": wc_b,
            "m0": m0, "bc": bc_b,
        })
    res = bass_utils.run_bass_kernel_spmd(
        nc, in_maps, core_ids=list(range(NCORES)), trace=_trace,
    )
    out = np.concatenate([r["logits"] for r in res.results], axis=0)
    if _trace:
        kernel.last_results = res
    return out


# revision 5
# speedup vs baseline: 1374.6598x; 1374.6598x over previous
"""MlssaSelector Trainium2 kernel (8-core SPMD, data-parallel over bags).

Pipeline per core (16384 sentences = 1024 bags, bag boundaries shard-aligned):
  sT  = W1.T @ xT            (PE, bf16, transposed domain: partitions = d/a)
  th  = tanh(sT)             (ACT, PSUM->SBUF, bf16 out)
  zT  = W2.T @ th            (PE)  -> [4 heads, n]
  e   = exp(zT)              (ACT; segment softmax needs no max-subtraction:
                              |z| <~ 3 so exp is safe in f32)
  den = segsum_16(e), rden = 1/den               (DVE, free-dim grouped reduce)
  wraw[n,b] = sum_h e[h,n] * rden[h,b]           (PE, K=4 matmul -> crosses w
                                                  from free-dim to partitions)
  wsel = 0.25 * bagmask * wraw                   (DVE; 0.25 folds head-mean)
  bagT[d,b] = x_nat.T-chunks @ wsel              (PE, x as stationary operand:
                                                  output is transposed directly)
  logits = bagT.T @ Wc + bc                      (PE + DVE bias add)
x is shipped in BOTH layouts as bf16 (16.75MB each) so total HBM traffic per
core stays at the 33.5MB f32-once roofline (~94us @ 358GB/s).
"""

import numpy as np
import ml_dtypes

import concourse.bacc as bacc
import concourse.mybir as mybir
import concourse.tile as tile
from concourse import bass_utils

BF16 = ml_dtypes.bfloat16

N = 131072
D = 512
A = 256          # D_ATT
H = 4            # heads
C = 53           # classes
BAG = 16
NCORES = 8
NSH = N // NCORES          # 16384 sentences per core
BSH = NSH // BAG           # 1024 bags per core
NT = 512                   # sentences per supertile
NSUP = NSH // NT           # 32 supertiles
SUBS = NT // 128           # 4 x 128-sentence subtiles per supertile
BAGS_PER_SUP = NT // BAG   # 32

_CACHE = {}


def _build_nc(repeat=1):
    nc = bacc.Bacc("TRN2", target_bir_lowering=False, debug=False)
    dt = mybir.dt

    xt_d = nc.dram_tensor("xt", [D, NSH], dt.bfloat16, kind="ExternalInput")
    xn_d = nc.dram_tensor("xn", [NSH, D], dt.bfloat16, kind="ExternalInput")
    w1_d = nc.dram_tensor("w1", [D, A], dt.bfloat16, kind="ExternalInput")
    w2_d = nc.dram_tensor("w2", [A, H], dt.bfloat16, kind="ExternalInput")
    wc_d = nc.dram_tensor("wc", [D, C], dt.bfloat16, kind="ExternalInput")
    m0_d = nc.dram_tensor("m0", [128, SUBS * 8], dt.float32, kind="ExternalInput")
    bc_d = nc.dram_tensor("bc", [128, C], dt.float32, kind="ExternalInput")
    out_d = nc.dram_tensor("logits", [BSH, C], dt.float32, kind="ExternalOutput")

    xt_r = xt_d[:].rearrange("(dc p) n -> p dc n", p=128)          # [128,4,NSH]
    xn_r = xn_d[:].rearrange("(j s p) d -> j p s d", p=128, s=SUBS)  # [32,128,4,512]
    out_r = out_d[:].rearrange("(t p) c -> p t c", p=128)          # [128,8,53]

    with tile.TileContext(nc) as tc:
        with (
            tc.tile_pool(name="consts", bufs=1) as consts,
            tc.tile_pool(name="xin", bufs=3) as xin,
            tc.tile_pool(name="acts", bufs=4) as acts,
            tc.tile_pool(name="small", bufs=3) as small,
            tc.tile_pool(name="persist", bufs=1) as persist,
            tc.tile_pool(name="ps_s", bufs=2, space="PSUM") as ps_s_pool,
            tc.tile_pool(name="ps_z", bufs=2, space="PSUM") as ps_z_pool,
            tc.tile_pool(name="ps_w", bufs=2, space="PSUM") as ps_w_pool,
            tc.tile_pool(name="ps_bag", bufs=2, space="PSUM") as ps_bag_pool,
        ):
            # ---- constants / weights ----
            w1_sb = consts.tile([128, 4, A], dt.bfloat16)
            nc.sync.dma_start(out=w1_sb, in_=w1_d[:].rearrange("(dc p) a -> p dc a", p=128))
            w2_sb = consts.tile([128, 2, H], dt.bfloat16)
            nc.sync.dma_start(out=w2_sb, in_=w2_d[:].rearrange("(ah p) h -> p ah h", p=128))
            wc_sb = consts.tile([128, 4, C], dt.bfloat16)
            nc.sync.dma_start(out=wc_sb, in_=wc_d[:].rearrange("(dc p) c -> p dc c", p=128))
            m0_sb = consts.tile([128, SUBS * 8], dt.float32)
            nc.sync.dma_start(out=m0_sb, in_=m0_d[:])
            bc_sb = consts.tile([128, C], dt.float32)
            nc.sync.dma_start(out=bc_sb, in_=bc_d[:])

            # transposed bag_repr accumulated over the whole shard: [d-part, dc, b]
            bagT = persist.tile([128, 4, BSH], dt.bfloat16)

            for _rep in range(repeat):
              for j in range(NSUP):
                xt_t = xin.tile([128, 4, NT], dt.bfloat16, tag="xt")
                nc.sync.dma_start(out=xt_t, in_=xt_r[:, :, j * NT:(j + 1) * NT])
                xn_t = xin.tile([128, SUBS, D], dt.bfloat16, tag="xn")
                nc.sync.dma_start(out=xn_t, in_=xn_r[j])

                # stage 1: sT = W1.T @ xT, tanh
                th_t = acts.tile([128, 2, NT], dt.bfloat16, tag="th")
                for ah in range(2):
                    ps_s = ps_s_pool.tile([128, NT], dt.float32, tag="s")
                    for dc in range(4):
                        nc.tensor.matmul(
                            ps_s,
                            w1_sb[:, dc, ah * 128:(ah + 1) * 128],
                            xt_t[:, dc, :],
                            start=dc == 0,
                            stop=dc == 3,
                        )
                    nc.scalar.activation(
                        th_t[:, ah, :], ps_s, mybir.ActivationFunctionType.Tanh
                    )

                # stage 2: zT = W2.T @ th  -> [4, NT]
                ps_z = ps_z_pool.tile([H, NT], dt.float32, tag="zz")
                for ah in range(2):
                    nc.tensor.matmul(
                        ps_z, w2_sb[:, ah, :], th_t[:, ah, :],
                        start=ah == 0, stop=ah == 1,
                    )

                # segment softmax (no max): e, den, rden
                e_t = acts.tile([H, NT], dt.bfloat16, tag="e")
                nc.scalar.activation(e_t, ps_z, mybir.ActivationFunctionType.Exp)
                den = small.tile([H, BAGS_PER_SUP], dt.float32, tag="den")
                nc.vector.tensor_reduce(
                    den, e_t.rearrange("h (b i) -> h b i", i=BAG),
                    axis=mybir.AxisListType.X, op=mybir.AluOpType.add,
                )
                rden = small.tile([H, BAGS_PER_SUP], dt.float32, tag="rden")
                nc.vector.reciprocal(rden, den)
                rdbf = small.tile([H, BAGS_PER_SUP], dt.bfloat16, tag="rdbf")
                nc.vector.tensor_copy(rdbf, rden)

                # wraw[n, b] = sum_h e[h, n] * rden[h, b]   (per 128-subtile)
                ps_w = ps_w_pool.tile([128, SUBS, 8], dt.float32, tag="w")
                for s in range(SUBS):
                    nc.tensor.matmul(
                        ps_w[:, s, :],
                        e_t[:, s * 128:(s + 1) * 128],
                        rdbf[:, s * 8:(s + 1) * 8],
                        start=True, stop=True,
                    )
                wsel = acts.tile([128, SUBS * 8], dt.bfloat16, tag="wsel")
                nc.vector.tensor_tensor(
                    out=wsel, in0=m0_sb,
                    in1=ps_w.rearrange("p s b -> p (s b)"),
                    op=mybir.AluOpType.mult,
                )

                # bagT[d, b] = sum_n x[n, d] * wsel[n, b]  (x stationary)
                ps_b = ps_bag_pool.tile([128, 4, BAGS_PER_SUP], dt.float32, tag="bag")
                for s in range(SUBS):
                    for dc in range(4):
                        nc.tensor.matmul(
                            ps_b[:, dc, 8 * s:8 * s + 8],
                            xn_t[:, s, dc * 128:(dc + 1) * 128],
                            wsel[:, 8 * s:8 * s + 8],
                            start=True, stop=True,
                        )
                nc.any.tensor_copy(bagT[:, :, j * BAGS_PER_SUP:(j + 1) * BAGS_PER_SUP], ps_b)

              # ---- tail: logits = bagT.T @ Wc + bc ----
              log_sb = persist.tile([128, BSH // 128, C], dt.float32, tag="log")
              for g in range(BSH // 128):
                ps_l = ps_z_pool.tile([128, C], dt.float32, tag="zz")
                for dc in range(4):
                    nc.tensor.matmul(
                        ps_l, bagT[:, dc, g * 128:(g + 1) * 128], wc_sb[:, dc, :],
                        start=dc == 0, stop=dc == 3,
                    )
                nc.vector.tensor_tensor(
                    out=log_sb[:, g, :], in0=ps_l, in1=bc_sb,
                    op=mybir.AluOpType.add,
                )
              nc.sync.dma_start(out=out_r, in_=log_sb)

    nc.compile()
    return nc


def _get_nc():
    if "nc" not in _CACHE:
        _CACHE["nc"] = _build_nc()
    return _CACHE["nc"]


def kernel(x, W1, W2, Wc, bc, seg_ids, n_bags, _trace=False):
    x = np.asarray(x, dtype=np.float32)
    W1 = np.asarray(W1, dtype=np.float32)
    W2 = np.asarray(W2, dtype=np.float32)
    Wc = np.asarray(Wc, dtype=np.float32)
    bc = np.asarray(bc, dtype=np.float32)

    w1_b = W1.astype(BF16)
    w2_b = W2.astype(BF16)
    wc_b = Wc.astype(BF16)
    bc_b = np.broadcast_to(bc, (128, C)).copy()
    # bag-membership mask scaled by 1/H (folds the head-mean)
    p = np.arange(128)
    m0 = np.zeros((128, SUBS * 8), np.float32)
    for s in range(SUBS):
        m0[np.arange(128), s * 8 + (p // BAG) % 8] = 0.25
    ident = np.eye(128, dtype=np.float32)

    nc = _get_nc()
    in_maps = []
    for c in range(NCORES):
        xs = x[c * NSH:(c + 1) * NSH]
        in_maps.append({
            "xt": np.ascontiguousarray(xs.T).astype(BF16),
            "xn": xs.astype(BF16),
            "w1": w1_b, "w2": w2_b, "wc": wc_b,
            "m0": m0, "bc": bc_b,
        })
    res = bass_utils.run_bass_kernel_spmd(
        nc, in_maps, core_ids=list(range(NCORES)), trace=_trace,
    )
    out = np.concatenate([r["logits"] for r in res.results], axis=0)
    if _trace:
        kernel.last_results = res
    return out
